# revision 18
# baseline (speedup 1.0000x reference)
"""Gaussian KDE (brute-force, bandwidth^2 = 1) on 8 Trainium2 NeuronCores.

Math:
    out_i = log( sum_j w_j * exp(-||x_i - y_j||^2 / 2) ) - (d/2) log(2pi) - log(sum_j w_j)
          = log( sum_j exp(x_i . y_j + b_j) ) - ||x_i||^2/2 - (d/2) log(2pi) - log(sum_j w_j)
    with b_j = log(w_j) - ||y_j||^2/2.

Device work per core (queries sharded 8-way, 512 queries/core):
    - scores = x . y + b via fp8e4m3 DoubleRow matmuls (99 effective K rows
      packed as [50, 2, n] pairs; x and y split hi/lo with power-of-2
      rescales to dodge fp8 denormals; 3-level fp8 bias expansion).
      The PE streams 1 output column/cycle at 1.2 GHz (HAM-capped), so the
      kernel is tensor-bound: everything else must stay off its critical
      path.
    - loop order: query-tile OUTER, supertile inner, with ALL moving data
      resident in SBUF (6.6 MB) -> the PE stationary changes only 4x.
    - drain/exp/sum of fp32 PSUM scores, split 2:1 across ACT and DVE so
      both run ~25% under the PE pace:
        * ACT tiles: table exp reading PSUM in place, row-sum fused via
          accum_out.
        * DVE tiles: bf16-Schraudolph - q = u16(C1B*s + C2B) whose bit
          pattern IS bf16(exp(s)) (u16 convert saturates, so s < -88
          cleanly underflows to +0.0), then a 2-level bf16 tensor_tensor
          tree (2x DVE mode) + short reduce for the row-sum.
    - final per query-tile (overlapped with the next tile's matmuls):
      reduce partials, ln via the inverse-Schraudolph bit trick on DVE,
      subtract the per-query constant.
"""

import numpy as np
import ml_dtypes

_Q, _N, _D = 4096, 65536, 32
_NCORES = 8
_QSHARD = _Q // _NCORES          # 512 queries per core
_QTILES = _QSHARD // 128         # 4 psum-partition tiles per core
_SUP = 2048                      # trains per supertile (4 psum banks)
_MMN = 512                       # moving free dim per matmul (1 psum bank)
_NSUP = _N // _SUP               # 32 supertiles
_KROWS = 99                      # 3*32 hi/lo cross rows + 3 bias rows
_KPAIR = 50                      # DoubleRow pairs (99 rows padded to 100)

_BF16 = ml_dtypes.bfloat16
_F8 = ml_dtypes.float8_e4m3fn

# fraction of drain units routed to the DVE Schraudolph path
_DVE_FRAC = 0.37
_UNIT = 1024                     # drain unit: 2 psum banks

# bf16 Schraudolph: bf16_bits(exp(s)) ~ u16(C1B*s + C2B)
_C1B = float(2 ** 7 / np.log(2.0))


def _c2b_mean_zero():
    """127*2^7 - delta*2^7 with delta tuned so the relative error of the
    linear-mantissa approximation has zero mean over uniform fractions."""
    f = (np.arange(100000, dtype=np.float64) + 0.5) / 100000.0
    m0 = np.mean((1.0 + f) * 2.0 ** (-f))
    m1 = np.mean(2.0 ** (-f))
    delta = (m0 - 1.0) / m1
    return float(127 * 2 ** 7 - delta * 2 ** 7)


_C2B = _c2b_mean_zero()

# inverse bit trick for the final ln: ln(S) ~ bits_f32(S)*(ln2/2^23) - C_LN,
# with the 0.0573 mean-zero offset for the linear-mantissa log approximation.
_LN_SCALE = float(np.log(2.0) / 2 ** 23)
_LN_OFF = float((127.0 - 0.057304959) * np.log(2.0))

_prog_cache: dict = {}


def _dve_tile_pattern(ntiles: int):
    """Bresenham assignment of drain tiles to the DVE path."""
    pat = []
    acc = 0.0
    for _ in range(ntiles):
        acc += _DVE_FRAC
        if acc >= 1.0:
            acc -= 1.0
            pat.append(True)
        else:
            pat.append(False)
    # keep the final drain (the kernel tail) on the cheaper ACT path
    pat[-1] = False
    return pat


def _build_program(n_trains: int):
    """Build the (identical-per-core) Bass program for n_trains train points."""
    import concourse.tile as tile
    from concourse import bacc, mybir

    f32 = mybir.dt.float32
    bf16 = mybir.dt.bfloat16
    u16 = mybir.dt.uint16
    i32 = mybir.dt.int32
    fp8 = mybir.dt.float8e4
    nsup = n_trains // _SUP

    nc = bacc.Bacc("TRN2", target_bir_lowering=False, debug=False,
                   num_devices=_NCORES)

    mv_d = nc.dram_tensor("mv", [_KPAIR, 2, n_trains], fp8,
                          kind="ExternalInput")
    st_d = nc.dram_tensor("st", [_KPAIR, 2, _QSHARD], fp8,
                          kind="ExternalInput")
    dv_d = nc.dram_tensor("dv", [128, _QTILES], f32, kind="ExternalInput")
    out_d = nc.dram_tensor("out", [128, _QTILES], f32, kind="ExternalOutput")

    nunit_sup = _SUP // _UNIT                    # drain units per supertile
    nunits = _QTILES * nsup * nunit_sup
    ncols = nsup * nunit_sup                     # partial-sum cols per qtile
    pat = _dve_tile_pattern(nunits)

    with tile.TileContext(nc) as tc:
        with (
            tc.tile_pool(name="const", bufs=1) as cpool,
            tc.tile_pool(name="mv", bufs=nsup) as mvpool,
            tc.tile_pool(name="q", bufs=3) as qpool,
            tc.tile_pool(name="small", bufs=2) as spool,
            tc.tile_pool(name="psum", bufs=4, space="PSUM") as ppool,
        ):
            st_sb = cpool.tile([_KPAIR, 2, _QSHARD], fp8)
            nc.sync.dma_start(st_sb[:], st_d[:])
            dv_sb = cpool.tile([128, _QTILES], f32)
            sall = cpool.tile([128, _QTILES * ncols], f32)
            fin = cpool.tile([128, _QTILES], f32)

            # all moving data resident in SBUF. chunk 0 is split into
            # bank-size pieces on the sync queue so the first matmul starts
            # as soon as the first 512 columns land; the remaining chunk
            # configs are issued from the otherwise-idle GpSimd sequencer so
            # they don't serialize behind chunk 0 on sync.
            mvs = []
            for s in range(nsup):
                mv_sb = mvpool.tile([_KPAIR, 2, _SUP], fp8)
                if s == 0:
                    for j in range(_SUP // _MMN):
                        sl = slice(j * _MMN, (j + 1) * _MMN)
                        nc.sync.dma_start(mv_sb[:, :, sl], mv_d[:, :, sl])
                else:
                    nc.gpsimd.dma_start(mv_sb[:],
                                        mv_d[:, :, s * _SUP:(s + 1) * _SUP])
                mvs.append(mv_sb)
            nc.gpsimd.dma_start(dv_sb[:], dv_d[:])

            for qt in range(_QTILES):
                st_qt = st_sb[:, :, qt * 128:(qt + 1) * 128]
                for s in range(nsup):
                    for u in range(nunit_sup):
                        base = u * _UNIT
                        ps = ppool.tile([128, _UNIT], f32)
                        for j in range(_UNIT // _MMN):
                            nc.tensor.matmul(
                                out=ps[:, j * _MMN:(j + 1) * _MMN],
                                lhsT=st_qt,
                                rhs=mvs[s][:, :, base + j * _MMN:
                                           base + (j + 1) * _MMN],
                                start=True, stop=True,
                                perf_mode=mybir.MatmulPerfMode.DoubleRow,
                            )
                        col = qt * ncols + s * nunit_sup + u
                        if pat[col]:
                            # DVE drain: Schraudolph u16 -> bf16 bits, then a
                            # 2-level bf16 add tree (2x mode) + short reduce
                            q = qpool.tile([128, _UNIT], u16)
                            nc.vector.tensor_scalar(
                                q[:], ps[:], _C1B, _C2B,
                                mybir.AluOpType.mult, mybir.AluOpType.add)
                            qb = q[:].bitcast(bf16)
                            h = _UNIT // 2
                            h1 = qpool.tile([128, h], bf16)
                            nc.vector.tensor_tensor(
                                h1[:], qb[:, 0:h], qb[:, h:2 * h],
                                mybir.AluOpType.add)
                            h2 = qpool.tile([128, h // 2], bf16)
                            nc.vector.tensor_tensor(
                                h2[:], h1[:, 0:h // 2], h1[:, h // 2:h],
                                mybir.AluOpType.add)
                            nc.vector.tensor_reduce(
                                sall[:, col:col + 1], h2[:],
                                axis=mybir.AxisListType.X,
                                op=mybir.AluOpType.add)
                        else:
                            nc.scalar.activation(
                                ps[:], ps[:],
                                mybir.ActivationFunctionType.Exp,
                                accum_out=sall[:, col:col + 1],
                            )

                # per-qtile final, overlapped with the next qtile's matmuls
                red = spool.tile([128, 1], f32)
                nc.vector.tensor_reduce(
                    red[:], sall[:, qt * ncols:(qt + 1) * ncols],
                    axis=mybir.AxisListType.X, op=mybir.AluOpType.add,
                )
                lnv = spool.tile([128, 1], f32)
                nc.vector.tensor_scalar(
                    lnv[:], red[:].bitcast(i32), _LN_SCALE, -_LN_OFF,
                    mybir.AluOpType.mult, mybir.AluOpType.add)
                nc.vector.tensor_sub(fin[:, qt:qt + 1], lnv[:],
                                     dv_sb[:, qt:qt + 1])
                nc.sync.dma_start(out_d[:, qt:qt + 1], fin[:, qt:qt + 1])

    nc.compile()
    return nc


def _get_program(n_trains: int):
    if n_trains not in _prog_cache:
        _prog_cache[n_trains] = _build_program(n_trains)
    return _prog_cache[n_trains]


def _f8(a):
    return np.asarray(a, dtype=np.float32).astype(_F8)


def _pack_rows(xrows, n):
    """[99, n] float32 rows -> [50, 2, n] fp8 DoubleRow layout (row r at
    [r//2, r%2]), padding row 99 with zeros."""
    full = np.zeros((_KPAIR * 2, n), dtype=_F8)
    full[:_KROWS] = _f8(xrows)
    return np.ascontiguousarray(full.reshape(_KPAIR, 2, n))


def _prep_inputs(X, X_train, sample_weight):
    X = np.ascontiguousarray(np.asarray(X, dtype=np.float32))
    Y = np.ascontiguousarray(np.asarray(X_train, dtype=np.float32))
    w = np.ascontiguousarray(np.asarray(sample_weight, dtype=np.float32))
    n = Y.shape[0]

    # per-train bias b_j = log w_j - ||y_j||^2/2 (terms below -35 are
    # utterly negligible; the clip keeps fp8 bias rows in range)
    w64 = w.astype(np.float64)
    b64 = np.log(np.maximum(w64, 1e-300)) - 0.5 * np.sum(
        Y.astype(np.float64) ** 2, axis=1)
    b64 = np.clip(b64, -35.0, None)

    # y side: hi/lo split with power-of-2 rescales (keep fp8 normals)
    yhi = _f8(Y)                                             # [n, 32]
    ylo8 = _f8((Y - yhi.astype(np.float32)) * 8.0)
    yhi8 = _f8(yhi.astype(np.float32) / 8.0)
    # bias: 3-level fp8 expansion  b ~ bhi + blo16/16 + blolo128/128
    bhi = _f8(b64)
    r1 = b64 - bhi.astype(np.float64)
    blo16 = _f8(r1 * 16.0)
    r2 = r1 - blo16.astype(np.float64) / 16.0
    blolo128 = _f8(r2 * 128.0)

    yrows = np.zeros((_KROWS, n), dtype=np.float32)
    yrows[0:32] = yhi.astype(np.float32).T
    yrows[32:64] = ylo8.astype(np.float32).T
    yrows[64:96] = yhi8.astype(np.float32).T
    yrows[96] = bhi.astype(np.float32)
    yrows[97] = blo16.astype(np.float32)
    yrows[98] = blolo128.astype(np.float32)
    mv = _pack_rows(yrows, n)

    # per-query constant: ||x||^2/2 + (d/2) log(2pi) + log(sum w)
    const = 0.5 * _D * np.log(2.0 * np.pi) + np.log(np.sum(w64))
    dv_all = (0.5 * np.sum(X.astype(np.float64) ** 2, axis=1)
              + const).astype(np.float32)  # [Q]

    in_maps = []
    for c in range(_NCORES):
        xq = X[c * _QSHARD:(c + 1) * _QSHARD]               # [512, 32]
        xhi = _f8(xq)
        xlo8 = _f8((xq - xhi.astype(np.float32)) * 8.0)
        xhi8 = _f8(xhi.astype(np.float32) / 8.0)
        xrows = np.zeros((_KROWS, _QSHARD), dtype=np.float32)
        xrows[0:32] = xhi.astype(np.float32).T
        xrows[32:64] = xhi8.astype(np.float32).T
        xrows[64:96] = xlo8.astype(np.float32).T
        xrows[96] = 1.0
        xrows[97] = 1.0 / 16.0
        xrows[98] = 1.0 / 128.0
        st = _pack_rows(xrows, _QSHARD)
        # dv laid out [128 partitions, qtiles]
        dv = np.ascontiguousarray(
            dv_all[c * _QSHARD:(c + 1) * _QSHARD].reshape(_QTILES, 128).T)
        in_maps.append({"mv": mv, "st": st, "dv": dv})
    return in_maps


def _gather(results):
    out = np.empty(_Q, dtype=np.float32)
    for c in range(_NCORES):
        res = results[c]["out"]                             # [128, QTILES]
        out[c * _QSHARD:(c + 1) * _QSHARD] = res.T.reshape(_QSHARD)
    return out


def kernel(X, X_train, sample_weight, _want_timing=False):
    from concourse.bass_utils import run_bass_kernel_spmd

    nc = _get_program(_N)
    in_maps = _prep_inputs(X, X_train, sample_weight)
    kres = run_bass_kernel_spmd(
        nc, in_maps, core_ids=list(range(_NCORES)),
        trace=bool(_want_timing),
    )
    out = _gather(kres.results)
    if _want_timing:
        return out, kres
    return out


# revision 19
# speedup vs baseline: 1.0259x; 1.0259x over previous
"""Gaussian KDE (brute-force, bandwidth^2 = 1) on 8 Trainium2 NeuronCores.

Math:
    out_i = log( sum_j w_j * exp(-||x_i - y_j||^2 / 2) ) - (d/2) log(2pi) - log(sum_j w_j)
          = log( sum_j exp(x_i . y_j + b_j) ) - ||x_i||^2/2 - (d/2) log(2pi) - log(sum_j w_j)
    with b_j = log(w_j) - ||y_j||^2/2.

Device work per core (queries sharded 8-way, 512 queries/core):
    - scores = x . y + b via fp8e4m3 DoubleRow matmuls (99 effective K rows
      packed as [50, 2, n] pairs; x and y split hi/lo with power-of-2
      rescales to dodge fp8 denormals; 3-level fp8 bias expansion).
      The PE streams 1 output column/cycle at 1.2 GHz (HAM-capped), so the
      kernel is tensor-bound: everything else must stay off its critical
      path.
    - loop order: query-tile OUTER, supertile inner, with ALL moving data
      resident in SBUF (6.6 MB) -> the PE stationary changes only 4x.
    - drain/exp/sum of fp32 PSUM scores, split 2:1 across ACT and DVE so
      both run ~25% under the PE pace:
        * ACT tiles: table exp reading PSUM in place, row-sum fused via
          accum_out.
        * DVE tiles: bf16-Schraudolph - q = u16(C1B*s + C2B) whose bit
          pattern IS bf16(exp(s)) (u16 convert saturates, so s < -88
          cleanly underflows to +0.0), then a 2-level bf16 tensor_tensor
          tree (2x DVE mode) + short reduce for the row-sum.
    - final per query-tile (overlapped with the next tile's matmuls):
      reduce partials, ln via the inverse-Schraudolph bit trick on DVE,
      subtract the per-query constant.
"""

import numpy as np
import ml_dtypes

_Q, _N, _D = 4096, 65536, 32
_NCORES = 8
_QSHARD = _Q // _NCORES          # 512 queries per core
_QTILES = _QSHARD // 128         # 4 psum-partition tiles per core
_SUP = 2048                      # trains per supertile (4 psum banks)
_MMN = 512                       # moving free dim per matmul (1 psum bank)
_NSUP = _N // _SUP               # 32 supertiles
_KROWS = 99                      # 3*32 hi/lo cross rows + 3 bias rows
_KPAIR = 50                      # DoubleRow pairs (99 rows padded to 100)

_BF16 = ml_dtypes.bfloat16
_F8 = ml_dtypes.float8_e4m3fn

# fraction of drain units routed to the DVE Schraudolph path
_DVE_FRAC = 0.37
_UNIT = 1024                     # drain unit: 2 psum banks

# bf16 Schraudolph: bf16_bits(exp(s)) ~ u16(C1B*s + C2B)
_C1B = float(2 ** 7 / np.log(2.0))


def _c2b_mean_zero():
    """127*2^7 - delta*2^7 with delta tuned so the relative error of the
    linear-mantissa approximation has zero mean over uniform fractions."""
    f = (np.arange(100000, dtype=np.float64) + 0.5) / 100000.0
    m0 = np.mean((1.0 + f) * 2.0 ** (-f))
    m1 = np.mean(2.0 ** (-f))
    delta = (m0 - 1.0) / m1
    return float(127 * 2 ** 7 - delta * 2 ** 7)


_C2B = _c2b_mean_zero()

# inverse bit trick for the final ln: ln(S) ~ bits_f32(S)*(ln2/2^23) - C_LN,
# with the 0.0573 mean-zero offset for the linear-mantissa log approximation.
_LN_SCALE = float(np.log(2.0) / 2 ** 23)
_LN_OFF = float((127.0 - 0.057304959) * np.log(2.0))

_prog_cache: dict = {}


def _dve_tile_pattern(ntiles: int):
    """Bresenham assignment of drain tiles to the DVE path."""
    pat = []
    acc = 0.0
    for _ in range(ntiles):
        acc += _DVE_FRAC
        if acc >= 1.0:
            acc -= 1.0
            pat.append(True)
        else:
            pat.append(False)
    # keep the final drain (the kernel tail) on the cheaper ACT path
    pat[-1] = False
    return pat


def _build_program(n_trains: int):
    """Build the (identical-per-core) Bass program for n_trains train points."""
    import concourse.tile as tile
    from concourse import bacc, mybir

    f32 = mybir.dt.float32
    bf16 = mybir.dt.bfloat16
    u16 = mybir.dt.uint16
    i32 = mybir.dt.int32
    fp8 = mybir.dt.float8e4
    nsup = n_trains // _SUP

    nc = bacc.Bacc("TRN2", target_bir_lowering=False, debug=False,
                   num_devices=_NCORES)

    mv_d = nc.dram_tensor("mv", [_KPAIR, 2, n_trains], fp8,
                          kind="ExternalInput")
    st_d = nc.dram_tensor("st", [_KPAIR, 2, _QSHARD], fp8,
                          kind="ExternalInput")
    dv_d = nc.dram_tensor("dv", [128, _QTILES], f32, kind="ExternalInput")
    out_d = nc.dram_tensor("out", [128, _QTILES], f32, kind="ExternalOutput")

    nunit_sup = _SUP // _UNIT                    # drain units per supertile
    nunits = _QTILES * nsup * nunit_sup
    ncols = nsup * nunit_sup                     # partial-sum cols per qtile
    pat = _dve_tile_pattern(nunits)

    with tile.TileContext(nc) as tc:
        with (
            tc.tile_pool(name="const", bufs=1) as cpool,
            tc.tile_pool(name="mv", bufs=nsup) as mvpool,
            tc.tile_pool(name="q", bufs=3) as qpool,
            tc.tile_pool(name="small", bufs=2) as spool,
            tc.tile_pool(name="psum", bufs=4, space="PSUM") as ppool,
        ):
            st_sb = cpool.tile([_KPAIR, 2, _QSHARD], fp8)
            nc.sync.dma_start(st_sb[:], st_d[:])
            dv_sb = cpool.tile([128, _QTILES], f32)
            nc.sync.dma_start(dv_sb[:], dv_d[:])
            sall = cpool.tile([128, _QTILES * ncols], f32)
            fin = cpool.tile([128, _QTILES], f32)

            # all moving data resident in SBUF; chunk DMAs issued upfront.
            # chunk 0 is split into bank-size pieces so the first matmul can
            # start as soon as the first 512 columns land.
            mvs = []
            for s in range(nsup):
                mv_sb = mvpool.tile([_KPAIR, 2, _SUP], fp8)
                if s == 0:
                    for j in range(_SUP // _MMN):
                        sl = slice(j * _MMN, (j + 1) * _MMN)
                        nc.sync.dma_start(mv_sb[:, :, sl], mv_d[:, :, sl])
                else:
                    nc.sync.dma_start(mv_sb[:],
                                      mv_d[:, :, s * _SUP:(s + 1) * _SUP])
                mvs.append(mv_sb)

            for qt in range(_QTILES):
                st_qt = st_sb[:, :, qt * 128:(qt + 1) * 128]
                for s in range(nsup):
                    for u in range(nunit_sup):
                        base = u * _UNIT
                        ps = ppool.tile([128, _UNIT], f32)
                        for j in range(_UNIT // _MMN):
                            nc.tensor.matmul(
                                out=ps[:, j * _MMN:(j + 1) * _MMN],
                                lhsT=st_qt,
                                rhs=mvs[s][:, :, base + j * _MMN:
                                           base + (j + 1) * _MMN],
                                start=True, stop=True,
                                perf_mode=mybir.MatmulPerfMode.DoubleRow,
                            )
                        col = qt * ncols + s * nunit_sup + u
                        if pat[col]:
                            # DVE drain: Schraudolph u16 -> bf16 bits, then a
                            # 2-level bf16 add tree (2x mode) + short reduce
                            q = qpool.tile([128, _UNIT], u16)
                            nc.vector.tensor_scalar(
                                q[:], ps[:], _C1B, _C2B,
                                mybir.AluOpType.mult, mybir.AluOpType.add)
                            qb = q[:].bitcast(bf16)
                            h = _UNIT // 2
                            h1 = qpool.tile([128, h], bf16)
                            nc.vector.tensor_tensor(
                                h1[:], qb[:, 0:h], qb[:, h:2 * h],
                                mybir.AluOpType.add)
                            h2 = qpool.tile([128, h // 2], bf16)
                            nc.vector.tensor_tensor(
                                h2[:], h1[:, 0:h // 2], h1[:, h // 2:h],
                                mybir.AluOpType.add)
                            nc.vector.tensor_reduce(
                                sall[:, col:col + 1], h2[:],
                                axis=mybir.AxisListType.X,
                                op=mybir.AluOpType.add)
                        else:
                            nc.scalar.activation(
                                ps[:], ps[:],
                                mybir.ActivationFunctionType.Exp,
                                accum_out=sall[:, col:col + 1],
                            )

                # per-qtile final, overlapped with the next qtile's matmuls
                red = spool.tile([128, 1], f32)
                nc.vector.tensor_reduce(
                    red[:], sall[:, qt * ncols:(qt + 1) * ncols],
                    axis=mybir.AxisListType.X, op=mybir.AluOpType.add,
                )
                lnv = spool.tile([128, 1], f32)
                nc.vector.tensor_scalar(
                    lnv[:], red[:].bitcast(i32), _LN_SCALE, -_LN_OFF,
                    mybir.AluOpType.mult, mybir.AluOpType.add)
                nc.vector.tensor_sub(fin[:, qt:qt + 1], lnv[:],
                                     dv_sb[:, qt:qt + 1])
                nc.sync.dma_start(out_d[:, qt:qt + 1], fin[:, qt:qt + 1])

    nc.compile()
    return nc


def _get_program(n_trains: int):
    if n_trains not in _prog_cache:
        _prog_cache[n_trains] = _build_program(n_trains)
    return _prog_cache[n_trains]


def _f8(a):
    return np.asarray(a, dtype=np.float32).astype(_F8)


def _pack_rows(xrows, n):
    """[99, n] float32 rows -> [50, 2, n] fp8 DoubleRow layout (row r at
    [r//2, r%2]), padding row 99 with zeros."""
    full = np.zeros((_KPAIR * 2, n), dtype=_F8)
    full[:_KROWS] = _f8(xrows)
    return np.ascontiguousarray(full.reshape(_KPAIR, 2, n))


def _prep_inputs(X, X_train, sample_weight):
    X = np.ascontiguousarray(np.asarray(X, dtype=np.float32))
    Y = np.ascontiguousarray(np.asarray(X_train, dtype=np.float32))
    w = np.ascontiguousarray(np.asarray(sample_weight, dtype=np.float32))
    n = Y.shape[0]

    # per-train bias b_j = log w_j - ||y_j||^2/2 (terms below -35 are
    # utterly negligible; the clip keeps fp8 bias rows in range)
    w64 = w.astype(np.float64)
    b64 = np.log(np.maximum(w64, 1e-300)) - 0.5 * np.sum(
        Y.astype(np.float64) ** 2, axis=1)
    b64 = np.clip(b64, -35.0, None)

    # y side: hi/lo split with power-of-2 rescales (keep fp8 normals)
    yhi = _f8(Y)                                             # [n, 32]
    ylo8 = _f8((Y - yhi.astype(np.float32)) * 8.0)
    yhi8 = _f8(yhi.astype(np.float32) / 8.0)
    # bias: 3-level fp8 expansion  b ~ bhi + blo16/16 + blolo128/128
    bhi = _f8(b64)
    r1 = b64 - bhi.astype(np.float64)
    blo16 = _f8(r1 * 16.0)
    r2 = r1 - blo16.astype(np.float64) / 16.0
    blolo128 = _f8(r2 * 128.0)

    yrows = np.zeros((_KROWS, n), dtype=np.float32)
    yrows[0:32] = yhi.astype(np.float32).T
    yrows[32:64] = ylo8.astype(np.float32).T
    yrows[64:96] = yhi8.astype(np.float32).T
    yrows[96] = bhi.astype(np.float32)
    yrows[97] = blo16.astype(np.float32)
    yrows[98] = blolo128.astype(np.float32)
    mv = _pack_rows(yrows, n)

    # per-query constant: ||x||^2/2 + (d/2) log(2pi) + log(sum w)
    const = 0.5 * _D * np.log(2.0 * np.pi) + np.log(np.sum(w64))
    dv_all = (0.5 * np.sum(X.astype(np.float64) ** 2, axis=1)
              + const).astype(np.float32)  # [Q]

    in_maps = []
    for c in range(_NCORES):
        xq = X[c * _QSHARD:(c + 1) * _QSHARD]               # [512, 32]
        xhi = _f8(xq)
        xlo8 = _f8((xq - xhi.astype(np.float32)) * 8.0)
        xhi8 = _f8(xhi.astype(np.float32) / 8.0)
        xrows = np.zeros((_KROWS, _QSHARD), dtype=np.float32)
        xrows[0:32] = xhi.astype(np.float32).T
        xrows[32:64] = xhi8.astype(np.float32).T
        xrows[64:96] = xlo8.astype(np.float32).T
        xrows[96] = 1.0
        xrows[97] = 1.0 / 16.0
        xrows[98] = 1.0 / 128.0
        st = _pack_rows(xrows, _QSHARD)
        # dv laid out [128 partitions, qtiles]
        dv = np.ascontiguousarray(
            dv_all[c * _QSHARD:(c + 1) * _QSHARD].reshape(_QTILES, 128).T)
        in_maps.append({"mv": mv, "st": st, "dv": dv})
    return in_maps


def _gather(results):
    out = np.empty(_Q, dtype=np.float32)
    for c in range(_NCORES):
        res = results[c]["out"]                             # [128, QTILES]
        out[c * _QSHARD:(c + 1) * _QSHARD] = res.T.reshape(_QSHARD)
    return out


def kernel(X, X_train, sample_weight, _want_timing=False):
    from concourse.bass_utils import run_bass_kernel_spmd

    nc = _get_program(_N)
    in_maps = _prep_inputs(X, X_train, sample_weight)
    kres = run_bass_kernel_spmd(
        nc, in_maps, core_ids=list(range(_NCORES)),
        trace=bool(_want_timing),
    )
    out = _gather(kres.results)
    if _want_timing:
        return out, kres
    return out


# revision 21
# speedup vs baseline: 1.0324x; 1.0063x over previous
"""Gaussian KDE (brute-force, bandwidth^2 = 1) on 8 Trainium2 NeuronCores.

Math:
    out_i = log( sum_j w_j * exp(-||x_i - y_j||^2 / 2) ) - (d/2) log(2pi) - log(sum_j w_j)
          = log( sum_j exp(x_i . y_j + b_j) ) - ||x_i||^2/2 - (d/2) log(2pi) - log(sum_j w_j)
    with b_j = log(w_j) - ||y_j||^2/2.

Device work per core (queries sharded 8-way, 512 queries/core):
    - scores = x . y + b via fp8e4m3 DoubleRow matmuls (99 effective K rows
      packed as [50, 2, n] pairs; x and y split hi/lo with power-of-2
      rescales to dodge fp8 denormals; 3-level fp8 bias expansion).
      The PE streams 1 output column/cycle at 1.2 GHz (HAM-capped), so the
      kernel is tensor-bound: everything else must stay off its critical
      path.
    - loop order: query-tile OUTER, supertile inner, with ALL moving data
      resident in SBUF (6.6 MB) -> the PE stationary changes only 4x.
    - drain/exp/sum of fp32 PSUM scores, split 2:1 across ACT and DVE so
      both run ~25% under the PE pace:
        * ACT tiles: table exp reading PSUM in place, row-sum fused via
          accum_out.
        * DVE tiles: bf16-Schraudolph - q = u16(C1B*s + C2B) whose bit
          pattern IS bf16(exp(s)) (u16 convert saturates, so s < -88
          cleanly underflows to +0.0), then a 2-level bf16 tensor_tensor
          tree (2x DVE mode) + short reduce for the row-sum.
    - final per query-tile (overlapped with the next tile's matmuls):
      reduce partials, ln via the inverse-Schraudolph bit trick on DVE,
      subtract the per-query constant.
"""

import numpy as np
import ml_dtypes

_Q, _N, _D = 4096, 65536, 32
_NCORES = 8
_QSHARD = _Q // _NCORES          # 512 queries per core
_QTILES = _QSHARD // 128         # 4 psum-partition tiles per core
_SUP = 2048                      # trains per supertile (4 psum banks)
_MMN = 512                       # moving free dim per matmul (1 psum bank)
_NSUP = _N // _SUP               # 32 supertiles
_KROWS = 99                      # 3*32 hi/lo cross rows + 3 bias rows
_KPAIR = 50                      # DoubleRow pairs (99 rows padded to 100)

_BF16 = ml_dtypes.bfloat16
_F8 = ml_dtypes.float8_e4m3fn

# fraction of drain units routed to the DVE Schraudolph path
_DVE_FRAC = 0.37
_UNIT = 1024                     # drain unit: 2 psum banks

# bf16 Schraudolph: bf16_bits(exp(s)) ~ u16(C1B*s + C2B)
_C1B = float(2 ** 7 / np.log(2.0))


def _c2b_mean_zero():
    """127*2^7 - delta*2^7 with delta tuned so the relative error of the
    linear-mantissa approximation has zero mean over uniform fractions."""
    f = (np.arange(100000, dtype=np.float64) + 0.5) / 100000.0
    m0 = np.mean((1.0 + f) * 2.0 ** (-f))
    m1 = np.mean(2.0 ** (-f))
    delta = (m0 - 1.0) / m1
    return float(127 * 2 ** 7 - delta * 2 ** 7)


_C2B = _c2b_mean_zero()

# inverse bit trick for the final ln: ln(S) ~ bits_f32(S)*(ln2/2^23) - C_LN,
# with the 0.0573 mean-zero offset for the linear-mantissa log approximation.
_LN_SCALE = float(np.log(2.0) / 2 ** 23)
_LN_OFF = float((127.0 - 0.057304959) * np.log(2.0))

_prog_cache: dict = {}


def _dve_tile_pattern(ntiles: int):
    """Bresenham assignment of drain tiles to the DVE path."""
    pat = []
    acc = 0.0
    for _ in range(ntiles):
        acc += _DVE_FRAC
        if acc >= 1.0:
            acc -= 1.0
            pat.append(True)
        else:
            pat.append(False)
    # keep the final drain (the kernel tail) on the cheaper ACT path
    pat[-1] = False
    return pat


def _build_program(n_trains: int):
    """Build the (identical-per-core) Bass program for n_trains train points."""
    import concourse.tile as tile
    from concourse import bacc, mybir

    f32 = mybir.dt.float32
    bf16 = mybir.dt.bfloat16
    u16 = mybir.dt.uint16
    i32 = mybir.dt.int32
    fp8 = mybir.dt.float8e4
    nsup = n_trains // _SUP

    nc = bacc.Bacc("TRN2", target_bir_lowering=False, debug=False,
                   num_devices=_NCORES)

    mv_d = nc.dram_tensor("mv", [_KPAIR, 2, n_trains], fp8,
                          kind="ExternalInput")
    st_d = nc.dram_tensor("st", [_KPAIR, 2, _QSHARD], fp8,
                          kind="ExternalInput")
    dv_d = nc.dram_tensor("dv", [128, _QTILES], f32, kind="ExternalInput")
    out_d = nc.dram_tensor("out", [128, _QTILES], f32, kind="ExternalOutput")

    nunit_sup = _SUP // _UNIT                    # drain units per supertile
    nunits = _QTILES * nsup * nunit_sup
    ncols = nsup * nunit_sup                     # partial-sum cols per qtile
    pat = _dve_tile_pattern(nunits)

    with tile.TileContext(nc) as tc:
        with (
            tc.tile_pool(name="const", bufs=1) as cpool,
            tc.tile_pool(name="mv", bufs=nsup) as mvpool,
            tc.tile_pool(name="q", bufs=3) as qpool,
            tc.tile_pool(name="small", bufs=2) as spool,
            tc.tile_pool(name="psum", bufs=4, space="PSUM") as ppool,
        ):
            st_sb = cpool.tile([_KPAIR, 2, _QSHARD], fp8)
            # qtile 0's stationary slice first: it gates the first matmul
            nc.sync.dma_start(st_sb[:, :, 0:128], st_d[:, :, 0:128])
            dv_sb = cpool.tile([128, _QTILES], f32)
            sall = cpool.tile([128, _QTILES * ncols], f32)
            fin = cpool.tile([128, _QTILES], f32)

            # all moving data resident in SBUF; chunk DMAs issued upfront.
            # chunk 0 is split into bank-size pieces so the first matmul can
            # start as soon as the first 512 columns land.
            mvs = []
            for s in range(nsup):
                mv_sb = mvpool.tile([_KPAIR, 2, _SUP], fp8)
                if s == 0:
                    for j in range(_SUP // _MMN):
                        sl = slice(j * _MMN, (j + 1) * _MMN)
                        nc.sync.dma_start(mv_sb[:, :, sl], mv_d[:, :, sl])
                else:
                    nc.sync.dma_start(mv_sb[:],
                                      mv_d[:, :, s * _SUP:(s + 1) * _SUP])
                mvs.append(mv_sb)
            nc.sync.dma_start(st_sb[:, :, 128:_QSHARD], st_d[:, :, 128:_QSHARD])
            nc.sync.dma_start(dv_sb[:], dv_d[:])

            for qt in range(_QTILES):
                st_qt = st_sb[:, :, qt * 128:(qt + 1) * 128]
                for s in range(nsup):
                    for u in range(nunit_sup):
                        base = u * _UNIT
                        ps = ppool.tile([128, _UNIT], f32)
                        for j in range(_UNIT // _MMN):
                            nc.tensor.matmul(
                                out=ps[:, j * _MMN:(j + 1) * _MMN],
                                lhsT=st_qt,
                                rhs=mvs[s][:, :, base + j * _MMN:
                                           base + (j + 1) * _MMN],
                                start=True, stop=True,
                                perf_mode=mybir.MatmulPerfMode.DoubleRow,
                            )
                        col = qt * ncols + s * nunit_sup + u
                        if pat[col]:
                            # DVE drain: Schraudolph u16 -> bf16 bits, then a
                            # 2-level bf16 add tree (2x mode) + short reduce
                            q = qpool.tile([128, _UNIT], u16)
                            nc.vector.tensor_scalar(
                                q[:], ps[:], _C1B, _C2B,
                                mybir.AluOpType.mult, mybir.AluOpType.add)
                            qb = q[:].bitcast(bf16)
                            h = _UNIT // 2
                            h1 = qpool.tile([128, h], bf16)
                            nc.vector.tensor_tensor(
                                h1[:], qb[:, 0:h], qb[:, h:2 * h],
                                mybir.AluOpType.add)
                            h2 = qpool.tile([128, h // 2], bf16)
                            nc.vector.tensor_tensor(
                                h2[:], h1[:, 0:h // 2], h1[:, h // 2:h],
                                mybir.AluOpType.add)
                            nc.vector.tensor_reduce(
                                sall[:, col:col + 1], h2[:],
                                axis=mybir.AxisListType.X,
                                op=mybir.AluOpType.add)
                        else:
                            nc.scalar.activation(
                                ps[:], ps[:],
                                mybir.ActivationFunctionType.Exp,
                                accum_out=sall[:, col:col + 1],
                            )

                # per-qtile final, overlapped with the next qtile's matmuls
                red = spool.tile([128, 1], f32)
                nc.vector.tensor_reduce(
                    red[:], sall[:, qt * ncols:(qt + 1) * ncols],
                    axis=mybir.AxisListType.X, op=mybir.AluOpType.add,
                )
                lnv = spool.tile([128, 1], f32)
                nc.vector.tensor_scalar(
                    lnv[:], red[:].bitcast(i32), _LN_SCALE, -_LN_OFF,
                    mybir.AluOpType.mult, mybir.AluOpType.add)
                nc.vector.tensor_sub(fin[:, qt:qt + 1], lnv[:],
                                     dv_sb[:, qt:qt + 1])

            nc.sync.dma_start(out_d[:], fin[:])

    nc.compile()
    return nc


def _get_program(n_trains: int):
    if n_trains not in _prog_cache:
        _prog_cache[n_trains] = _build_program(n_trains)
    return _prog_cache[n_trains]


def _f8(a):
    return np.asarray(a, dtype=np.float32).astype(_F8)


def _pack_rows(xrows, n):
    """[99, n] float32 rows -> [50, 2, n] fp8 DoubleRow layout (row r at
    [r//2, r%2]), padding row 99 with zeros."""
    full = np.zeros((_KPAIR * 2, n), dtype=_F8)
    full[:_KROWS] = _f8(xrows)
    return np.ascontiguousarray(full.reshape(_KPAIR, 2, n))


def _prep_inputs(X, X_train, sample_weight):
    X = np.ascontiguousarray(np.asarray(X, dtype=np.float32))
    Y = np.ascontiguousarray(np.asarray(X_train, dtype=np.float32))
    w = np.ascontiguousarray(np.asarray(sample_weight, dtype=np.float32))
    n = Y.shape[0]

    # per-train bias b_j = log w_j - ||y_j||^2/2 (terms below -35 are
    # utterly negligible; the clip keeps fp8 bias rows in range)
    w64 = w.astype(np.float64)
    b64 = np.log(np.maximum(w64, 1e-300)) - 0.5 * np.sum(
        Y.astype(np.float64) ** 2, axis=1)
    b64 = np.clip(b64, -35.0, None)

    # y side: hi/lo split with power-of-2 rescales (keep fp8 normals)
    yhi = _f8(Y)                                             # [n, 32]
    ylo8 = _f8((Y - yhi.astype(np.float32)) * 8.0)
    yhi8 = _f8(yhi.astype(np.float32) / 8.0)
    # bias: 3-level fp8 expansion  b ~ bhi + blo16/16 + blolo128/128
    bhi = _f8(b64)
    r1 = b64 - bhi.astype(np.float64)
    blo16 = _f8(r1 * 16.0)
    r2 = r1 - blo16.astype(np.float64) / 16.0
    blolo128 = _f8(r2 * 128.0)

    yrows = np.zeros((_KROWS, n), dtype=np.float32)
    yrows[0:32] = yhi.astype(np.float32).T
    yrows[32:64] = ylo8.astype(np.float32).T
    yrows[64:96] = yhi8.astype(np.float32).T
    yrows[96] = bhi.astype(np.float32)
    yrows[97] = blo16.astype(np.float32)
    yrows[98] = blolo128.astype(np.float32)
    mv = _pack_rows(yrows, n)

    # per-query constant: ||x||^2/2 + (d/2) log(2pi) + log(sum w)
    const = 0.5 * _D * np.log(2.0 * np.pi) + np.log(np.sum(w64))
    dv_all = (0.5 * np.sum(X.astype(np.float64) ** 2, axis=1)
              + const).astype(np.float32)  # [Q]

    in_maps = []
    for c in range(_NCORES):
        xq = X[c * _QSHARD:(c + 1) * _QSHARD]               # [512, 32]
        xhi = _f8(xq)
        xlo8 = _f8((xq - xhi.astype(np.float32)) * 8.0)
        xhi8 = _f8(xhi.astype(np.float32) / 8.0)
        xrows = np.zeros((_KROWS, _QSHARD), dtype=np.float32)
        xrows[0:32] = xhi.astype(np.float32).T
        xrows[32:64] = xhi8.astype(np.float32).T
        xrows[64:96] = xlo8.astype(np.float32).T
        xrows[96] = 1.0
        xrows[97] = 1.0 / 16.0
        xrows[98] = 1.0 / 128.0
        st = _pack_rows(xrows, _QSHARD)
        # dv laid out [128 partitions, qtiles]
        dv = np.ascontiguousarray(
            dv_all[c * _QSHARD:(c + 1) * _QSHARD].reshape(_QTILES, 128).T)
        in_maps.append({"mv": mv, "st": st, "dv": dv})
    return in_maps


def _gather(results):
    out = np.empty(_Q, dtype=np.float32)
    for c in range(_NCORES):
        res = results[c]["out"]                             # [128, QTILES]
        out[c * _QSHARD:(c + 1) * _QSHARD] = res.T.reshape(_QSHARD)
    return out


def kernel(X, X_train, sample_weight, _want_timing=False):
    from concourse.bass_utils import run_bass_kernel_spmd

    nc = _get_program(_N)
    in_maps = _prep_inputs(X, X_train, sample_weight)
    kres = run_bass_kernel_spmd(
        nc, in_maps, core_ids=list(range(_NCORES)),
        trace=bool(_want_timing),
    )
    out = _gather(kres.results)
    if _want_timing:
        return out, kres
    return out
